# revision 12
# baseline (speedup 1.0000x reference)
"""AttentionBlock (GroupNorm + single-head self-attention + proj + residual)
Trainium2 Bass/Tile kernel, data-parallel over batch across 8 NeuronCores.

Reference computation (per batch element b of 16; C=512, H=W=32, N=1024):
  h   = GroupNorm(x, 8 groups, eps=1e-5) * gn_scale + gn_bias
  qkv = qkv_w @ h + qkv_b            (1x1 conv == matmul over channels)
  q,k,v = split(qkv); attn = softmax(q^T k / sqrt(C)); o = v @ attn^T
  y   = proj_w @ o + proj_b + x

Layout strategy per core (2 batch elements per core):
  - Everything channel-major [C(part-tiles), N(free)] so all matmuls contract
    over the 128-partition dim with no transposes:
      Q,K  : [c, n]  via lhsT = qkv_w^T column tiles
      V^T  : [n, c]  via lhsT = h n-subtiles, rhs = Wv^T
      S^T  : [m(keys), n(queries)] via lhsT = K m-subtiles, rhs = Q
      E    = exp(S^T / sqrt(C)) (no max-subtract needed: logits ~ N(0,1))
      denom: ones^T @ E (PE column-sum), reciprocal on DVE, broadcast back to
             128 partitions with a K=1 matmul
      O    : [c, n] via lhsT = V^T c-subtiles, rhs = E; scaled by recip on evict
      y    : [c, n] via lhsT = proj_w^T, rhs = O; + residual x on evict
  - K-bias is dropped: it shifts every logit of a query equally, which softmax
    cancels exactly. V-bias is folded into the proj bias on the host.
  - Matmul inputs bf16 (PE full rate), all accumulation fp32 in PSUM,
    GroupNorm stats + softmax denominators fp32.
"""

import sys

for _p in ("/opt/trn_rl_repo",):
    if _p not in sys.path:
        sys.path.insert(0, _p)

import math

import ml_dtypes
import numpy as np

import concourse.bass as bass
import concourse.tile as tile
from concourse import mybir
from concourse.vector_clock import ScopedClock, VectorClock

B, C, H, W = 16, 512, 32, 32
N = H * W  # 1024
NUM_GROUPS = 8
EPS = 1e-5
N_CORES = 8
NB = B // N_CORES  # batches per core = 2
CT = C // 128  # channel partition tiles = 4
NT = N // 128  # pixel partition tiles = 8
NH = N // 512  # free-dim halves = 2
GSIZE = C // NUM_GROUPS  # 64 channels per group
SCALE = 1.0 / math.sqrt(C)

F32 = mybir.dt.float32
BF16 = mybir.dt.bfloat16
BF16_NP = ml_dtypes.bfloat16


# --- workaround: this container's walrus accepts only ONE sync wait on the
# SP CTRL drain that TileContext emits at kernel tail; split it into
# single-wait drains.
def _chunked_drain_and_barrier(self, tick_clock, wait_clock):
    gc = tick_clock.global_clock
    ticks = None
    for _k, v in ScopedClock({None: gc}).items():
        ticks = eval(repr(v).replace("VectorClock", ""))
    assert ticks is not None
    n = len(ticks)
    for i in range(n):
        if ticks[i] <= 0:
            continue
        cticks = [ticks[j] if j == i else 0 for j in range(n)]
        drain_inst = self.nc.sync.drain()
        wait_clock.add_sem_waits(
            drain_inst.ins, ScopedClock({None: VectorClock(cticks)})
        )
    self.nc.all_engine_barrier()
    popped = self.nc._tile_sem_poison_stack.pop()
    assert popped is self._sem_poison
    self.nc.clear_and_free_semaphores(list(self.sems.allocated().values()))
    self.nc.all_engine_barrier()


tile.TileContext._drain_and_barrier = _chunked_drain_and_barrier


def _split_multi_waits(nc: bass.Bass, max_waits: int = 1) -> None:
    """Walrus in this container rejects instructions carrying more than one
    sync wait. Hoist excess waits onto same-engine NoOp carriers placed
    immediately before the instruction (same engine queue -> same blocking
    semantics)."""
    n_split = 0
    for f in nc.m.functions:
        for bb in f.blocks:
            insts = bb.instructions
            new = []
            for inst in insts:
                si = inst.sync_info
                if si is not None and len(si.on_wait) > max_waits:
                    waits = list(si.on_wait)
                    keep = waits[-max_waits:]
                    for w in waits[: -max_waits]:
                        nop = mybir.InstNoOp(
                            name=f"{inst.name}-wsplit{n_split}",
                            engine=inst.engine,
                            bass_nofuse=True,
                            sync_info=mybir.SyncInfo(on_wait=[w], on_update=[]),
                        )
                        new.append(nop)
                        n_split += 1
                    inst.sync_info = mybir.SyncInfo(
                        on_wait=keep, on_update=list(si.on_update)
                    )
                new.append(inst)
            insts[:] = new


def build_nc(q_bias_nonzero: bool, p_bias_nonzero: bool) -> bass.Bass:
    nc = bass.Bass(trn_type="TRN2")

    x_d = nc.dram_tensor("x", [NB, C, N], F32, kind="ExternalInput")
    wqkvT_d = nc.dram_tensor("wqkvT", [C, 3 * C], BF16, kind="ExternalInput")
    pwT_d = nc.dram_tensor("pwT", [C, C], BF16, kind="ExternalInput")
    # packed per-c-tile vectors: [gnsc, gnbi, qb, pb2, gmat(8 cols)]
    vecs_d = nc.dram_tensor(
        "vecs", [CT, 128, 4 + NUM_GROUPS], F32, kind="ExternalInput"
    )
    gmatT_d = nc.dram_tensor("gmatT", [NUM_GROUPS, C], F32, kind="ExternalInput")
    y_d = nc.dram_tensor("y", [NB, C, N], F32, kind="ExternalOutput")

    xap = x_d.ap()
    yap = y_d.ap()

    with tile.TileContext(nc) as tc:
        with (
            tc.tile_pool(name="singles", bufs=1) as singles,
            tc.tile_pool(name="xin", bufs=2) as xin,
            tc.tile_pool(name="stats", bufs=2) as stats,
            tc.tile_pool(name="hp", bufs=2) as hp,
            tc.tile_pool(name="qk", bufs=2) as qkp,
            tc.tile_pool(name="vt", bufs=2) as vtp,
            tc.tile_pool(name="ep", bufs=2) as ep,
            tc.tile_pool(name="op", bufs=2) as opl,
            tc.tile_pool(name="yp", bufs=4) as ypl,
            tc.tile_pool(name="ps_mm", bufs=3, space="PSUM") as ps_mm,
            tc.tile_pool(name="ps_o", bufs=2, space="PSUM") as ps_o,
            tc.tile_pool(name="ps_aux", bufs=2, space="PSUM") as ps_aux,
            tc.tile_pool(name="ps_warm", bufs=1, space="PSUM") as ps_warm,
            tc.tile_pool(name="dscratch", bufs=2, space="DRAM") as dscratch,
        ):
            # ---- x loads first (they gate GN stats -> everything); HWDGE on
            # the idle sync engine so SWDGE descriptor-gen doesn't serialize
            # the startup. Also run per-partition bn_stats as tiles arrive.
            xt_all = [[None] * CT for _ in range(NB)]
            mq_all = [[None] * CT for _ in range(NB)]
            for b in range(NB):
                for ct in range(CT):
                    t = xin.tile([128, N], F32, tag=f"x{b}_{ct}")
                    nc.sync.dma_start(
                        out=t, in_=xap[b, ct * 128 : (ct + 1) * 128, :]
                    )
                    xt_all[b][ct] = t
                    st = stats.tile([128, 2, 6], F32, tag=f"st{ct}")
                    for sub in range(2):
                        nc.vector.bn_stats(
                            out=st[:, sub, :], in_=t[:, sub * 512 : (sub + 1) * 512]
                        )
                    m = stats.tile([128, 3], F32, tag=f"mq{b}_{ct}")
                    nc.vector.bn_aggr(out=m[:, 0:2], in_=st)
                    # col2 = mean^2 (for cross-partition variance aggregation)
                    nc.vector.tensor_mul(m[:, 2:3], m[:, 0:1], m[:, 0:1])
                    mq_all[b][ct] = m

            # ---- constants / weights (resident); SWDGE on gpsimd, packed to
            # few DMAs. vecs packs [gnsc, gnbi, qb, pb2, gmat(8)] per c-tile.
            wt_sb = []
            pw_sb = []
            vecs_sb = []
            for ct in range(CT):
                w = singles.tile([128, 3 * C], BF16, tag=f"wqkv{ct}")
                nc.gpsimd.dma_start(out=w, in_=wqkvT_d.ap()[ct * 128 : (ct + 1) * 128, :])
                wt_sb.append(w)
                p = singles.tile([128, C], BF16, tag=f"pw{ct}")
                nc.gpsimd.dma_start(out=p, in_=pwT_d.ap()[ct * 128 : (ct + 1) * 128, :])
                pw_sb.append(p)
                v = singles.tile([128, 4 + NUM_GROUPS], F32, tag=f"vecs{ct}")
                nc.gpsimd.dma_start(out=v, in_=vecs_d.ap()[ct])
                vecs_sb.append(v)
            gnsc_sb = [v[:, 0:1] for v in vecs_sb]
            gnbi_sb = [v[:, 1:2] for v in vecs_sb]
            qb_sb = [v[:, 2:3] for v in vecs_sb]
            pb2_sb = [v[:, 3:4] for v in vecs_sb]
            gm_sb = [v[:, 4 : 4 + NUM_GROUPS] for v in vecs_sb]
            gmT_full = singles.tile([NUM_GROUPS, C], F32, tag="gmT")
            nc.gpsimd.dma_start(out=gmT_full, in_=gmatT_d.ap())
            gmT_sb = [
                gmT_full[:, ct * 128 : (ct + 1) * 128] for ct in range(CT)
            ]
            ones_bf = singles.tile([128, 1], BF16, tag="ones_bf")
            nc.vector.memset(ones_bf, 1.0)
            eps_t = singles.tile([NUM_GROUPS, 1], F32, tag="eps")
            nc.vector.memset(eps_t, EPS)

            # ---- PE warm-up: HAM unthrottles after ~3.4us of sustained PE
            # activity; the first real matmul can only start once GN stats
            # are in (~8-10us). Burn the wait on dummy matmuls so real work
            # runs at 2.4GHz from the first instruction.
            warm_rhs = singles.tile([128, 512], BF16, tag="warm_rhs")
            nc.vector.memset(warm_rhs, 0.0)
            warm_ps = ps_warm.tile([1, 512], F32, tag="warm")
            for wi in range(28):
                nc.tensor.matmul(
                    warm_ps, lhsT=ones_bf, rhs=warm_rhs, start=True, stop=True
                )

            for b in range(NB):
                xt = xt_all[b]
                mq = mq_all[b]

                # group stats: [8, 3] = [mean_g, E var_p, E mean_p^2] (gmat holds 1/64)
                gst_ps = ps_aux.tile([NUM_GROUPS, 3], F32, tag="aux")
                for ct in range(CT):
                    nc.tensor.matmul(
                        gst_ps,
                        lhsT=gm_sb[ct],
                        rhs=mq[ct],
                        start=(ct == 0),
                        stop=(ct == CT - 1),
                    )
                gs3 = stats.tile([NUM_GROUPS, 3], F32, tag="gs3")
                nc.vector.tensor_copy(out=gs3, in_=gst_ps)
                var = stats.tile([NUM_GROUPS, 1], F32, tag="var")
                m2 = stats.tile([NUM_GROUPS, 1], F32, tag="m2")
                nc.vector.tensor_add(var, gs3[:, 1:2], gs3[:, 2:3])
                nc.vector.tensor_mul(m2, gs3[:, 0:1], gs3[:, 0:1])
                nc.vector.tensor_sub(var, var, m2)
                # var -> 1/sqrt(var+eps)
                nc.scalar.activation(
                    out=var, in_=var, func=mybir.ActivationFunctionType.Sqrt,
                    bias=eps_t, scale=1.0,
                )
                nc.vector.reciprocal(out=var, in_=var)
                st2 = stats.tile([NUM_GROUPS, 2], F32, tag="st2")
                nc.vector.tensor_copy(out=st2[:, 0:1], in_=gs3[:, 0:1])
                nc.vector.tensor_copy(out=st2[:, 1:2], in_=var)

                # ---------- apply GN -> h (bf16, channel-major) ----------
                ht = []
                for ct in range(CT):
                    bc_ps = ps_aux.tile([128, 2], F32, tag="aux")
                    nc.tensor.matmul(
                        bc_ps, lhsT=gmT_sb[ct], rhs=st2, start=True, stop=True
                    )
                    A = stats.tile([128, 1], F32, tag=f"A{ct}")
                    Bt = stats.tile([128, 1], F32, tag=f"B{ct}")
                    nc.vector.tensor_mul(A, bc_ps[:, 1:2], gnsc_sb[ct])
                    nc.vector.tensor_mul(Bt, bc_ps[:, 0:1], A)
                    nc.vector.tensor_sub(Bt, gnbi_sb[ct], Bt)
                    h = hp.tile([128, N], BF16, tag=f"h{ct}")
                    nc.vector.tensor_scalar(
                        out=h, in0=xt[ct], scalar1=A, scalar2=Bt,
                        op0=mybir.AluOpType.mult, op1=mybir.AluOpType.add,
                    )
                    ht.append(h)

                # ---------- Q, K (channel-major) ----------
                q_sb = []
                k_sb = []
                for qk, off, lst in (("q", 0, q_sb), ("k", C, k_sb)):
                    for co in range(CT):
                        dst = qkp.tile([128, N], BF16, tag=f"{qk}{co}")
                        for nh in range(NH):
                            ps = ps_mm.tile([128, 512], F32, tag="mm")
                            for ct in range(CT):
                                nc.tensor.matmul(
                                    ps,
                                    lhsT=wt_sb[ct][:, off + co * 128 : off + (co + 1) * 128],
                                    rhs=ht[ct][:, nh * 512 : (nh + 1) * 512],
                                    start=(ct == 0),
                                    stop=(ct == CT - 1),
                                )
                            dslice = dst[:, nh * 512 : (nh + 1) * 512]
                            if qk == "q" and q_bias_nonzero:
                                nc.scalar.activation(
                                    out=dslice, in_=ps,
                                    func=mybir.ActivationFunctionType.Identity,
                                    bias=qb_sb[co],
                                )
                            else:
                                nc.scalar.copy(out=dslice, in_=ps)
                        lst.append(dst)

                # ---------- V^T : [n, c] ----------
                vt_sb = []
                for nt in range(NT):
                    ps = ps_mm.tile([128, 512], F32, tag="mm")
                    for ct in range(CT):
                        nc.tensor.matmul(
                            ps,
                            lhsT=ht[ct][:, nt * 128 : (nt + 1) * 128],
                            rhs=wt_sb[ct][:, 2 * C : 3 * C],
                            start=(ct == 0),
                            stop=(ct == CT - 1),
                        )
                    vt = vtp.tile([128, 512], BF16, tag=f"vt{nt}")
                    nc.scalar.copy(out=vt, in_=ps)
                    vt_sb.append(vt)

                # ---------- attention ----------
                for nh in range(NH):
                    es = []
                    dps = ps_aux.tile([1, 512], F32, tag="aux")
                    for mt in range(NT):
                        sps = ps_mm.tile([128, 512], F32, tag="mm")
                        for ck in range(CT):
                            nc.tensor.matmul(
                                sps,
                                lhsT=k_sb[ck][:, mt * 128 : (mt + 1) * 128],
                                rhs=q_sb[ck][:, nh * 512 : (nh + 1) * 512],
                                start=(ck == 0),
                                stop=(ck == CT - 1),
                            )
                        e = ep.tile([128, 512], BF16, tag=f"e{mt}")
                        nc.scalar.activation(
                            out=e, in_=sps,
                            func=mybir.ActivationFunctionType.Exp, scale=SCALE,
                        )
                        es.append(e)
                        # denominator: column-sums of E accumulated over m-tiles
                        nc.tensor.matmul(
                            dps, lhsT=ones_bf, rhs=e,
                            start=(mt == 0), stop=(mt == NT - 1),
                        )

                    # O (channel-major), unnormalized
                    o_ps = []
                    for ct4 in range(CT):
                        ops_ = ps_o.tile([128, 512], F32, tag="o")
                        for mt in range(NT):
                            nc.tensor.matmul(
                                ops_,
                                lhsT=vt_sb[mt][:, ct4 * 128 : (ct4 + 1) * 128],
                                rhs=es[mt],
                                start=(mt == 0),
                                stop=(mt == NT - 1),
                            )
                        o_ps.append(ops_)

                    # 1/denominator, replicated to 128 partitions via a DRAM
                    # round-trip broadcast DMA (keeps it off the PE; an SBUF
                    # source may not have partition step 0, a DRAM source may)
                    rd = stats.tile([1, 512], F32, tag="rd")
                    nc.vector.reciprocal(out=rd, in_=dps)
                    rd_dram = dscratch.tile([1, 512], F32, tag="rd_dram")
                    nc.gpsimd.dma_start(out=rd_dram, in_=rd)
                    rb = stats.tile([128, 512], F32, tag="rb_sb")
                    nc.gpsimd.dma_start(out=rb, in_=rd_dram.to_broadcast([128, 512]))

                    o_sb = []
                    for ct4 in range(CT):
                        o = opl.tile([128, 512], BF16, tag=f"o{ct4}")
                        nc.vector.tensor_mul(o, o_ps[ct4], rb)
                        o_sb.append(o)

                    # ---------- proj + residual ----------
                    for cot in range(CT):
                        yps = ps_mm.tile([128, 512], F32, tag="mm")
                        for ct4 in range(CT):
                            nc.tensor.matmul(
                                yps,
                                lhsT=pw_sb[ct4][:, cot * 128 : (cot + 1) * 128],
                                rhs=o_sb[ct4],
                                start=(ct4 == 0),
                                stop=(ct4 == CT - 1),
                            )
                        yo = ypl.tile([128, 512], F32, tag="y")
                        if p_bias_nonzero:
                            nc.scalar.activation(
                                out=yo, in_=yps,
                                func=mybir.ActivationFunctionType.Identity,
                                bias=pb2_sb[cot],
                            )
                            nc.vector.tensor_add(
                                yo, yo, xt[cot][:, nh * 512 : (nh + 1) * 512]
                            )
                        else:
                            nc.vector.tensor_add(
                                yo, yps, xt[cot][:, nh * 512 : (nh + 1) * 512]
                            )
                        nc.sync.dma_start(
                            out=yap[b, cot * 128 : (cot + 1) * 128, nh * 512 : (nh + 1) * 512],
                            in_=yo,
                        )
    _split_multi_waits(nc)
    return nc


_NC_CACHE: dict = {}


def _get_nc(q_bias_nonzero: bool, p_bias_nonzero: bool) -> bass.Bass:
    key = (q_bias_nonzero, p_bias_nonzero)
    if key not in _NC_CACHE:
        _NC_CACHE[key] = build_nc(*key)
    return _NC_CACHE[key]


def kernel(x, gn_scale, gn_bias, qkv_w, qkv_b, proj_w, proj_b, _trace=False):
    from concourse.bass_utils import run_bass_kernel_spmd

    x = np.asarray(x, dtype=np.float32)
    gn_scale = np.asarray(gn_scale, dtype=np.float32)
    gn_bias = np.asarray(gn_bias, dtype=np.float32)
    qkv_w = np.asarray(qkv_w, dtype=np.float32)
    qkv_b = np.asarray(qkv_b, dtype=np.float32)
    proj_w = np.asarray(proj_w, dtype=np.float32)
    proj_b = np.asarray(proj_b, dtype=np.float32)

    qb = qkv_b[:C]
    vb = qkv_b[2 * C : 3 * C]
    # K-bias is softmax-invariant (constant per-query logit shift) -> dropped.
    # V-bias passes linearly through attention (weights sum to 1) -> fold into
    # the proj bias.
    pb2 = proj_w @ vb + proj_b

    q_bias_nonzero = bool(np.any(qb != 0))
    p_bias_nonzero = bool(np.any(pb2 != 0))
    nc = _get_nc(q_bias_nonzero, p_bias_nonzero)

    wqkvT = np.ascontiguousarray(qkv_w.T).astype(BF16_NP)
    pwT = np.ascontiguousarray(proj_w.T).astype(BF16_NP)

    ch = np.arange(C)
    grp = ch // GSIZE  # group id per channel
    gmat = np.zeros((C, NUM_GROUPS), np.float32)
    gmat[ch, grp] = 1.0 / (GSIZE * 1.0)  # mean over 64 partition-stats
    gmatT = np.zeros((NUM_GROUPS, C), np.float32)
    gmatT[grp, ch] = 1.0

    vecs = np.concatenate(
        [
            gn_scale.reshape(C, 1),
            gn_bias.reshape(C, 1),
            qb.reshape(C, 1),
            pb2.reshape(C, 1).astype(np.float32),
            gmat,
        ],
        axis=1,
    ).reshape(CT, 128, 4 + NUM_GROUPS)

    xr = x.reshape(B, C, N)
    shared = {
        "wqkvT": wqkvT,
        "pwT": pwT,
        "vecs": np.ascontiguousarray(vecs),
        "gmatT": gmatT,
    }
    in_maps = [
        {"x": np.ascontiguousarray(xr[c * NB : (c + 1) * NB]), **shared}
        for c in range(N_CORES)
    ]
    res = run_bass_kernel_spmd(
        nc, in_maps, core_ids=list(range(N_CORES)), trace=_trace
    )
    y = np.concatenate([res.results[c]["y"] for c in range(N_CORES)], axis=0)
    out = y.reshape(B, C, H, W).astype(np.float32)
    if _trace:
        return out, res
    return out
